# revision 23
# baseline (speedup 1.0000x reference)
"""Distributed multi-head attention kernel for one TRN2 chip (8 NeuronCores).

Problem: nn_Attention_13048110645268
  x [2, 2048, 1024] f32 ->  attention(16 heads, d=64) -> out [2, 2048, 1024] f32

Sharding (Megatron-style batch x head-group):
  core c in [0,8): batch b = c//4, head group g = c%4 (heads 4g..4g+3).
  Each core computes qkv projections for its 4 heads, attention for those
  heads, then all-gathers the (unprojected) attention outputs within its
  4-core batch group and computes a 256-column slice of the output
  projection.  Host reassembles the full output (pure layout ops).

Per-core device pipeline (all matmuls bf16, accumulation fp32):
  qkT  [512,2048]  = (Wqk)^T x^T + bias      (feature-major)
  v    [2048,256]  = x Wv                    (token-major, lhsT = x^T tile)
  per query block qb (512 queries) / key tile kt (128 keys):
      S^T[kt,qt]   = k q^T  (per head, 2 heads row-packed, K=64)
      E = exp(S*scale) on ScalarE (PSUM->SBUF bf16), 2 instrs of [128,1024]
      out'^T[d,qt] += lhsT=v[kt,64], rhs=E^T  (2 heads col-packed)
      rowsum[qt]   += ones^T E^T  (4 heads col-packed, M=1)
      (av/rowsum of key tile kt run one iteration behind the scores of
       kt+1 so the PE stays busy while ScalarE computes exp)
  normalize: out^T = out'^T * (1/rowsum) broadcast via small DRAM roundtrip
  AllGather out^T [256,512] -> [1024,512] per qb (replica groups [0-3],[4-7])
  yT[256,2048] = Wp^T outT_full + beff  (fp32 output)

Host pre-restripes all weight/activation inputs so every big DMA is a
plain [128, N]-contiguous transfer (cheap descriptor generation).
"""

import os
import sys

import numpy as np

sys.path.insert(0, "/opt/trn_rl_repo")

import ml_dtypes  # noqa: E402

import concourse.bass as bass  # noqa: E402
import concourse.mybir as mybir  # noqa: E402
import concourse.tile as tile  # noqa: E402
from concourse import bacc  # noqa: E402
from concourse.bass_utils import run_bass_kernel_spmd  # noqa: E402

BF16 = mybir.dt.bfloat16
F32 = mybir.dt.float32
NBF16 = ml_dtypes.bfloat16

B, S, D = 2, 2048, 1024
H, HD = 16, 64
NCORES = 8
GROUPS = [[0, 1, 2, 3], [4, 5, 6, 7]]
HL = 4          # heads per core
DL = HL * HD    # 256 feature dims per core
P = 128
KT = S // P     # 16 key tiles
QB = 4          # query blocks
QW = S // QB    # 512 queries per block
KD = D // P     # 8 contraction tiles over model dim
SCALE = HD ** -0.5

_CACHE = {}


def _restripe(w):
    """[KD*128, C] -> [128, KD*C] with row p holding all kd-subtiles."""
    kd = w.shape[0] // P
    return np.ascontiguousarray(
        w.reshape(kd, P, w.shape[1]).transpose(1, 0, 2).reshape(P, -1))


def _emit(nc: bass.Bass, tc: tile.TileContext, xT, wqk, wv, wp, bqk, beff, yT):
    exp_fn = mybir.ActivationFunctionType.Exp

    with (
        tc.tile_pool(name="main", bufs=1) as mp,
        tc.tile_pool(name="ep", bufs=4) as ep,
        tc.tile_pool(name="gp", bufs=2) as gp,
        tc.tile_pool(name="yp", bufs=2) as yp,
        tc.tile_pool(name="rp", bufs=2) as rp,
        tc.tile_pool(name="ps_s", bufs=1, space="PSUM") as ps_s,
        tc.tile_pool(name="ps_acc", bufs=3, space="PSUM") as ps_acc,
        tc.tile_pool(name="ps_mm", bufs=1, space="PSUM") as ps_mm,
        tc.tile_pool(name="dram", bufs=2, space="DRAM") as dp,
    ):
        # ---------------- input DMA (ordered by first use) ----------------
        wqk_sb = mp.tile([P, KD, 2 * DL], BF16)
        nc.sync.dma_start(wqk_sb[:],
                          wqk[:, :].rearrange("p (kd c) -> p kd c", kd=KD))
        bqk_sb = mp.tile([P, 4], F32)
        nc.sync.dma_start(bqk_sb[:], bqk[:, :])
        xT_sb = mp.tile([P, QB, KD, 512], BF16)   # x^T [d-part, n, d-tile, tok]
        nc.sync.dma_start(xT_sb[:, 0],
                          xT[0, :, :].rearrange("p (kd u) -> p kd u", kd=KD))
        wv_sb = mp.tile([P, KD, DL], BF16)
        nc.sync.dma_start(wv_sb[:],
                          wv[:, :].rearrange("p (kd c) -> p kd c", kd=KD))
        for n in range(1, QB):
            nc.sync.dma_start(xT_sb[:, n],
                              xT[n, :, :].rearrange("p (kd u) -> p kd u", kd=KD))
        wp_sb = mp.tile([P, KD, DL], BF16)
        nc.sync.dma_start(wp_sb[:],
                          wp[:, :].rearrange("p (kd c) -> p kd c", kd=KD))
        beff_sb = mp.tile([P, 2], F32)
        nc.sync.dma_start(beff_sb[:], beff[:, :])
        ones_sb = mp.tile([P, 1], BF16)
        nc.vector.memset(ones_sb[:], 1.0)
        onesf_sb = mp.tile([P, 64], F32)
        nc.vector.memset(onesf_sb[:], 1.0)

        # ---------------- qk projection: qkT_sb[c, t] ----------------
        # ct 0,1 = q (heads 0..3), ct 2,3 = k (heads 0..3).  Only the n=0
        # block is emitted up front; the rest is interleaved into attention
        # (deadline-scheduled) so ScalarE starts exp as early as possible.
        qkT_sb = mp.tile([P, 4, S], BF16)

        def emit_qk(n, ct, pre=False):
            # pre-loop groups pipeline through the 3 'acc' slots (free until
            # the first av/rs allocation); in-loop groups must use the
            # rotating 'mm' slot to avoid deadlocking against the qb-long
            # accumulator tiles.
            if pre:
                ps_qk = ps_acc.tile([P, 512], F32, tag="acc", name="ps_qk")
            else:
                ps_qk = ps_mm.tile([P, 512], F32, tag="mm", name="ps_qk")
            for kd in range(KD):
                nc.tensor.matmul(
                    ps_qk[:],
                    lhsT=wqk_sb[:, kd, ct * P:(ct + 1) * P],
                    rhs=xT_sb[:, n, kd, :],
                    start=(kd == 0),
                    stop=(kd == KD - 1),
                )
            nc.vector.tensor_scalar_add(
                qkT_sb[:, ct, n * 512:(n + 1) * 512], ps_qk[:],
                bqk_sb[:, ct:ct + 1],
            )

        # PE warm-up: dummy matmuls with no input deps run while the input
        # DMAs land, lifting the HAM clock gate to 8/8 before real work
        warm_sb = mp.tile([P, 512], BF16)
        nc.vector.memset(warm_sb[:], 1.0)
        ps_warm = ps_s.tile([P, 4 * 512], F32, name="ps_warm", tag="sc")
        for w in range(48):
            nc.tensor.matmul(
                ps_warm[:, (w % 4) * 512:(w % 4 + 1) * 512],
                lhsT=warm_sb[:, 0:P],
                rhs=warm_sb[:, :],
                start=True,
                stop=True,
            )

        for ct in (2, 3, 0, 1):
            emit_qk(0, ct, pre=True)

        # ---------------- attention + AG + proj, per query block ----------------
        v_sb = mp.tile([P, KT, DL], BF16)
        outT_sb = mp.tile([P, 2, S], BF16)   # pair j: heads 2j (p<64), 2j+1
        g_tiles = [None] * QB

        _vpair = [None]

        def emit_v(tt):
            if tt % 2 == 0:
                _vpair[0] = ps_mm.tile([P, 512], F32, tag="mm", name="ps_v")
            half = (tt % 2) * DL
            ps_v = _vpair[0]
            for kd in range(KD):
                nc.tensor.matmul(
                    ps_v[:, half:half + DL],
                    lhsT=xT_sb[:, tt // 4, kd, (tt % 4) * P:(tt % 4 + 1) * P],
                    rhs=wv_sb[:, kd, :],
                    start=(kd == 0),
                    stop=(kd == KD - 1),
                )
            nc.vector.tensor_copy(v_sb[:, tt, :], ps_v[:, half:half + DL])

        def emit_proj_half(qb, j):
            qs = qb * QW
            g_sb = g_tiles[qb]
            ps_y = ps_mm.tile([P, 512], F32, tag="mm", name="ps_y")
            for kd in range(KD):
                nc.tensor.matmul(
                    ps_y[:],
                    lhsT=wp_sb[:, kd, j * P:(j + 1) * P],
                    rhs=g_sb[:, kd, :],
                    start=(kd == 0),
                    stop=(kd == KD - 1),
                )
            y_sb = yp.tile([P, 512], F32, name="y_sb")
            nc.vector.tensor_scalar_add(y_sb[:], ps_y[:], beff_sb[:, j:j + 1])
            nc.sync.dma_start(yT[j * P:(j + 1) * P, qs:qs + QW], y_sb[:])

        def emit_av_pair(kt, e_sb, ps_av, pair):
            for hh in range(2):
                h = 2 * pair + hh
                nc.tensor.matmul(
                    ps_av[64 * hh:64 * hh + HD, :],
                    lhsT=v_sb[:, kt, h * HD:(h + 1) * HD],
                    rhs=e_sb[:, h * 512:(h + 1) * 512],
                    start=(kt == 0),
                    stop=(kt == KT - 1),
                )

        def emit_rs(kt, e_sb, ps_rs):
            for h in range(HL):
                nc.tensor.matmul(
                    ps_rs[32 * h:32 * h + 1, :],
                    lhsT=ones_sb[:, 0:1],
                    rhs=e_sb[:, h * 512:(h + 1) * 512],
                    start=(kt == 0),
                    stop=(kt == KT - 1),
                    tile_position=(0, 32 * h),
                )

        def make_norm_pair(qb, j, o_sb, r_sb):
            qs = qb * QW

            def _norm():
                rb_ps = ps_mm.tile([P, 512], F32, tag="mm", name="rb_ps")
                for hh in range(2):
                    h = 2 * j + hh
                    nc.tensor.matmul(
                        rb_ps[64 * hh:64 * hh + 64, :],
                        lhsT=onesf_sb[32 * h:32 * h + 1, :],
                        rhs=r_sb[32 * h:32 * h + 1, :],
                        start=True,
                        stop=True,
                        tile_position=(32 * h, 64 * hh),
                    )
                nc.vector.tensor_mul(outT_sb[:, j, qs:qs + QW], o_sb[:],
                                     rb_ps[:])
            return _norm

        def make_ag(qb):
            qs = qb * QW

            def _ag():
                cc_in = dp.tile([2 * P, QW], BF16, name="cc_in")
                nc.sync.dma_start(cc_in[:, :].rearrange("(j p) t -> p j t", p=P),
                                  outT_sb[:, :, qs:qs + QW])
                cc_out = dp.tile([D, QW], BF16, name="cc_out")
                nc.gpsimd.collective_compute(
                    "AllGather",
                    mybir.AluOpType.bypass,
                    replica_groups=GROUPS,
                    ins=[cc_in[:, :].opt()],
                    outs=[cc_out[:, :].opt()],
                )
                g_sb = gp.tile([P, KD, QW], BF16, name="g_sb")
                nc.sync.dma_start(
                    g_sb[:], cc_out[:, :].rearrange("(kd p) t -> p kd t", p=P))
                g_tiles[qb] = g_sb
            return _ag

        # Deadline-scheduled PE filler for each (qb, kt) iteration:
        #  - qb0 carries the remaining qk blocks (k tiles via the acc pool
        #    before the lag-3 accumulators are allocated) and all v tiles
        #  - qb>=1 carry the q blocks for later qbs, the normalization +
        #    AllGather of qb-1 (kt1/kt2), and proj of qb-1 (kt12/kt14)
        filler = {qb: {} for qb in range(QB)}

        def _add(qb, kt, fn):
            filler[qb].setdefault(kt, []).append(fn)

        _add(0, 0, lambda: emit_qk(1, 2, pre=True))
        _add(0, 1, lambda: emit_qk(1, 3, pre=True))
        _add(0, 1, lambda: emit_qk(2, 2, pre=True))
        _add(0, 2, lambda: emit_qk(2, 3, pre=True))
        _add(0, 2, lambda: emit_qk(3, 2, pre=True))
        _add(0, 2, lambda: emit_qk(3, 3, pre=True))
        _v_sched = {_t: [_t] for _t in range(12)}
        _v_sched[11] = [11, 12]
        _v_sched[12] = [13, 14]
        _v_sched[13] = [15]
        for _kt, _ts in _v_sched.items():
            for _t in _ts:
                _add(0, _kt, lambda t=_t: emit_v(t))
        _add(0, 9, lambda: emit_qk(1, 0))
        _add(0, 12, lambda: emit_qk(1, 1))
        _add(1, 2, lambda: emit_qk(2, 0))
        _add(1, 5, lambda: emit_qk(2, 1))
        _add(2, 2, lambda: emit_qk(3, 0))
        _add(2, 5, lambda: emit_qk(3, 1))
        for _qb in (2, 3):
            _add(_qb, 8, lambda q=_qb: emit_proj_half(q - 2, 0))
            _add(_qb, 10, lambda q=_qb: emit_proj_half(q - 2, 1))

        def emit_scores_pair(ps_sc, qb, kt, pair):
            qs = qb * QW
            for hh in range(2):
                h = 2 * pair + hh
                hp = (HD * h) % P                 # 0, 64, 0, 64
                hc = h // 2                       # q ctile; k ctile = 2 + hc
                nc.tensor.matmul(
                    ps_sc[:, h * 512:(h + 1) * 512],
                    lhsT=qkT_sb[hp:hp + HD, 2 + hc, kt * P:(kt + 1) * P],
                    rhs=qkT_sb[hp:hp + HD, hc, qs:qs + QW],
                    start=True,
                    stop=True,
                )

        sc_next = None
        for qb in range(QB):
            qs = qb * QW
            lag = 3 if qb == 0 else 0
            ps_av0 = ps_av1 = ps_rs = None
            pending = []

            if sc_next is None:
                sc_next = ps_s.tile([P, 4 * 512], F32, name="ps_sc", tag="sc")
                emit_scores_pair(sc_next, qb, 0, 0)
                emit_scores_pair(sc_next, qb, 0, 1)
            sc_cur = sc_next
            sc_next = None

            for kt in range(KT):
                e_sb = ep.tile([P, 4 * 512], BF16, name="e_sb")
                nc.scalar.activation(e_sb[:, 0:1024], sc_cur[:, 0:1024], exp_fn,
                                     scale=SCALE)
                nc.scalar.activation(e_sb[:, 1024:2048], sc_cur[:, 1024:2048],
                                     exp_fn, scale=SCALE)
                nxt = kt + 1 < KT
                if nxt:
                    sc_nx = ps_s.tile([P, 4 * 512], F32, name="ps_sc", tag="sc")
                if lag == 0:
                    if ps_av0 is None:
                        ps_av0 = ps_acc.tile([P, 512], F32, tag="acc",
                                             name="ps_av0")
                        ps_av1 = ps_acc.tile([P, 512], F32, tag="acc",
                                             name="ps_av1")
                        ps_rs = ps_acc.tile([P, 512], F32, tag="acc",
                                            name="ps_rs")
                    if nxt:
                        emit_scores_pair(sc_nx, qb, kt + 1, 0)
                    emit_av_pair(kt, e_sb, ps_av0, 0)
                    if nxt:
                        emit_scores_pair(sc_nx, qb, kt + 1, 1)
                    emit_av_pair(kt, e_sb, ps_av1, 1)
                    emit_rs(kt, e_sb, ps_rs)
                else:
                    if nxt:
                        emit_scores_pair(sc_nx, qb, kt + 1, 0)
                        emit_scores_pair(sc_nx, qb, kt + 1, 1)
                    pending.append((kt, e_sb))
                    if len(pending) > lag:
                        if ps_av0 is None:
                            ps_av0 = ps_acc.tile([P, 512], F32, tag="acc",
                                                 name="ps_av0")
                            ps_av1 = ps_acc.tile([P, 512], F32, tag="acc",
                                                 name="ps_av1")
                            ps_rs = ps_acc.tile([P, 512], F32, tag="acc",
                                                name="ps_rs")
                        k0, e0 = pending.pop(0)
                        emit_av_pair(k0, e0, ps_av0, 0)
                        emit_av_pair(k0, e0, ps_av1, 1)
                        emit_rs(k0, e0, ps_rs)
                for fn in filler[qb].get(kt, ()):
                    fn()
                if nxt:
                    sc_cur = sc_nx
            for k0, e0 in pending:
                emit_av_pair(k0, e0, ps_av0, 0)
                emit_av_pair(k0, e0, ps_av1, 1)
                emit_rs(k0, e0, ps_rs)

            if qb + 1 < QB:
                sc_next = ps_s.tile([P, 4 * 512], F32, name="ps_sc", tag="sc")
                emit_scores_pair(sc_next, qb + 1, 0, 0)
                emit_scores_pair(sc_next, qb + 1, 0, 1)

            # release the accumulator PSUM slots fast: raw copies to SBUF
            o_sb = [rp.tile([P, 512], BF16, name="o0_sb"),
                    rp.tile([P, 512], BF16, name="o1_sb")]
            nc.vector.tensor_copy(o_sb[0][:], ps_av0[:])
            nc.vector.tensor_copy(o_sb[1][:], ps_av1[:])
            r_sb = rp.tile([P, 512], F32, name="r_sb")
            nc.vector.reciprocal(r_sb[:], ps_rs[:])   # rows 0/32/64/96 valid

            if qb < QB - 1:
                _add(qb + 1, 1, make_norm_pair(qb, 0, o_sb[0], r_sb))
                _add(qb + 1, 2, make_norm_pair(qb, 1, o_sb[1], r_sb))
                _add(qb + 1, 2, make_ag(qb))
            else:
                make_norm_pair(qb, 0, o_sb[0], r_sb)()
                make_norm_pair(qb, 1, o_sb[1], r_sb)()
                make_ag(qb)()

        emit_proj_half(QB - 2, 0)
        emit_proj_half(QB - 2, 1)
        emit_proj_half(QB - 1, 0)
        emit_proj_half(QB - 1, 1)


def _build():
    if "nc" in _CACHE:
        return _CACHE["nc"]
    nc = bacc.Bacc(
        "TRN2",
        target_bir_lowering=False,
        debug=False,
        num_devices=NCORES,
    )
    xT = nc.declare_dram_parameter("xT", [QB, P, KD * 512], BF16, isOutput=False)
    wqk = nc.declare_dram_parameter("wqk", [P, KD * 2 * DL], BF16, isOutput=False)
    wv = nc.declare_dram_parameter("wv", [P, KD * DL], BF16, isOutput=False)
    wp = nc.declare_dram_parameter("wp", [P, KD * DL], BF16, isOutput=False)
    bqk = nc.declare_dram_parameter("bqk", [P, 4], F32, isOutput=False)
    beff = nc.declare_dram_parameter("beff", [P, 2], F32, isOutput=False)
    yT = nc.declare_dram_parameter("yT", [DL, S], F32, isOutput=True)

    with tile.TileContext(nc) as tc:
        _emit(nc, tc, xT, wqk, wv, wp, bqk, beff, yT)
    nc.compile()
    _CACHE["nc"] = nc
    return nc


def kernel(x, W_qkv, b_qkv, W_proj, b_proj):
    x = np.asarray(x, np.float32)
    W_qkv = np.asarray(W_qkv, np.float32)
    b_qkv = np.asarray(b_qkv, np.float32)
    W_proj = np.asarray(W_proj, np.float32)
    b_proj = np.asarray(b_proj, np.float32)

    nc = _build()

    b_v = b_qkv[2 * D:3 * D]
    xTt = {}
    for b in range(B):
        xT = np.ascontiguousarray(x[b].T)            # [1024, 2048]
        t = xT.reshape(KD, P, QB, 512).transpose(2, 1, 0, 3)
        xTt[b] = np.ascontiguousarray(t.reshape(QB, P, KD * 512)).astype(NBF16)

    in_maps = []
    for c in range(NCORES):
        b, g = divmod(c, 4)
        cs = DL * g
        wqk_c = np.concatenate(
            [W_qkv[:, cs:cs + DL], W_qkv[:, D + cs:D + cs + DL]], axis=1)
        bqk_c = np.concatenate(
            [b_qkv[cs:cs + DL], b_qkv[D + cs:D + cs + DL]]).reshape(4, P).T
        beff_c = (b_v @ W_proj[:, cs:cs + DL] + b_proj[cs:cs + DL]).reshape(2, P).T
        in_maps.append({
            "xT": xTt[b],
            "wqk": _restripe(wqk_c).astype(NBF16),
            "wv": _restripe(W_qkv[:, 2 * D + cs:2 * D + cs + DL]).astype(NBF16),
            "wp": _restripe(W_proj[:, cs:cs + DL]).astype(NBF16),
            "bqk": np.ascontiguousarray(bqk_c, np.float32),
            "beff": np.ascontiguousarray(beff_c, np.float32),
        })

    trace = bool(int(os.environ.get("TRN_KERNEL_TRACE", "0")))
    res = run_bass_kernel_spmd(nc, in_maps, core_ids=list(range(NCORES)),
                               trace=trace)
    if trace and res.exec_time_ns is not None:
        print(f"HW exec time: {res.exec_time_ns} ns", flush=True)
    _CACHE["last_result"] = res

    out = np.empty((B, S, D), np.float32)
    for c in range(NCORES):
        b, g = divmod(c, 4)
        out[b, :, DL * g:DL * (g + 1)] = res.results[c]["yT"].T
    return out


# revision 24
# speedup vs baseline: 1.0165x; 1.0165x over previous
"""Distributed multi-head attention kernel for one TRN2 chip (8 NeuronCores).

Problem: nn_Attention_13048110645268
  x [2, 2048, 1024] f32 ->  attention(16 heads, d=64) -> out [2, 2048, 1024] f32

Sharding (Megatron-style batch x head-group):
  core c in [0,8): batch b = c//4, head group g = c%4 (heads 4g..4g+3).
  Each core computes qkv projections for its 4 heads, attention for those
  heads, then all-gathers the (unprojected) attention outputs within its
  4-core batch group and computes a 256-column slice of the output
  projection.  Host reassembles the full output (pure layout ops).

Per-core device pipeline (all matmuls bf16, accumulation fp32):
  qkT  [512,2048]  = (Wqk)^T x^T + bias      (feature-major)
  v    [2048,256]  = x Wv                    (token-major, lhsT = x^T tile)
  per query block qb (512 queries) / key tile kt (128 keys):
      S^T[kt,qt]   = k q^T  (per head, 2 heads row-packed, K=64)
      E = exp(S*scale) on ScalarE (PSUM->SBUF bf16), 2 instrs of [128,1024]
      out'^T[d,qt] += lhsT=v[kt,64], rhs=E^T  (2 heads col-packed)
      rowsum[qt]   += ones^T E^T  (4 heads col-packed, M=1)
      (av/rowsum of key tile kt run one iteration behind the scores of
       kt+1 so the PE stays busy while ScalarE computes exp)
  normalize: out^T = out'^T * (1/rowsum) broadcast via small DRAM roundtrip
  AllGather out^T [256,512] -> [1024,512] per qb (replica groups [0-3],[4-7])
  yT[256,2048] = Wp^T outT_full + beff  (fp32 output)

Host pre-restripes all weight/activation inputs so every big DMA is a
plain [128, N]-contiguous transfer (cheap descriptor generation).
"""

import os
import sys

import numpy as np

sys.path.insert(0, "/opt/trn_rl_repo")

import ml_dtypes  # noqa: E402

import concourse.bass as bass  # noqa: E402
import concourse.mybir as mybir  # noqa: E402
import concourse.tile as tile  # noqa: E402
from concourse import bacc  # noqa: E402
from concourse.bass_utils import run_bass_kernel_spmd  # noqa: E402

BF16 = mybir.dt.bfloat16
F32 = mybir.dt.float32
NBF16 = ml_dtypes.bfloat16

B, S, D = 2, 2048, 1024
H, HD = 16, 64
NCORES = 8
GROUPS = [[0, 1, 2, 3], [4, 5, 6, 7]]
HL = 4          # heads per core
DL = HL * HD    # 256 feature dims per core
P = 128
KT = S // P     # 16 key tiles
QB = 4          # query blocks
QW = S // QB    # 512 queries per block
KD = D // P     # 8 contraction tiles over model dim
SCALE = HD ** -0.5

_CACHE = {}


def _restripe(w):
    """[KD*128, C] -> [128, KD*C] with row p holding all kd-subtiles."""
    kd = w.shape[0] // P
    return np.ascontiguousarray(
        w.reshape(kd, P, w.shape[1]).transpose(1, 0, 2).reshape(P, -1))


def _emit(nc: bass.Bass, tc: tile.TileContext, xT, wqk, wv, wp, bqk, beff, yT):
    exp_fn = mybir.ActivationFunctionType.Exp

    with (
        tc.tile_pool(name="main", bufs=1) as mp,
        tc.tile_pool(name="ep", bufs=4) as ep,
        tc.tile_pool(name="gp", bufs=2) as gp,
        tc.tile_pool(name="yp", bufs=2) as yp,
        tc.tile_pool(name="rp", bufs=2) as rp,
        tc.tile_pool(name="ps_s", bufs=1, space="PSUM") as ps_s,
        tc.tile_pool(name="ps_acc", bufs=3, space="PSUM") as ps_acc,
        tc.tile_pool(name="ps_mm", bufs=1, space="PSUM") as ps_mm,
        tc.tile_pool(name="dram", bufs=2, space="DRAM") as dp,
    ):
        # ---------------- input DMA (ordered by first use) ----------------
        wqk_sb = mp.tile([P, KD, 2 * DL], BF16)
        nc.sync.dma_start(wqk_sb[:],
                          wqk[:, :].rearrange("p (kd c) -> p kd c", kd=KD))
        bqk_sb = mp.tile([P, 4], F32)
        nc.sync.dma_start(bqk_sb[:], bqk[:, :])
        xT_sb = mp.tile([P, QB, KD, 512], BF16)   # x^T [d-part, n, d-tile, tok]
        nc.sync.dma_start(xT_sb[:, 0],
                          xT[0, :, :].rearrange("p (kd u) -> p kd u", kd=KD))
        wv_sb = mp.tile([P, KD, DL], BF16)
        nc.sync.dma_start(wv_sb[:],
                          wv[:, :].rearrange("p (kd c) -> p kd c", kd=KD))
        for n in range(1, QB):
            nc.sync.dma_start(xT_sb[:, n],
                              xT[n, :, :].rearrange("p (kd u) -> p kd u", kd=KD))
        wp_sb = mp.tile([P, KD, DL], BF16)
        nc.sync.dma_start(wp_sb[:],
                          wp[:, :].rearrange("p (kd c) -> p kd c", kd=KD))
        beff_sb = mp.tile([P, 2], F32)
        nc.sync.dma_start(beff_sb[:], beff[:, :])
        ones_sb = mp.tile([P, 1], BF16)
        nc.vector.memset(ones_sb[:], 1.0)
        onesf_sb = mp.tile([P, 64], F32)
        nc.vector.memset(onesf_sb[:], 1.0)

        # ---------------- qk projection: qkT_sb[c, t] ----------------
        # ct 0,1 = q (heads 0..3), ct 2,3 = k (heads 0..3).  Only the n=0
        # block is emitted up front; the rest is interleaved into attention
        # (deadline-scheduled) so ScalarE starts exp as early as possible.
        qkT_sb = mp.tile([P, 4, S], BF16)

        def emit_qk(n, ct, pre=False):
            # pre-loop groups pipeline through the 3 'acc' slots (free until
            # the first av/rs allocation); in-loop groups must use the
            # rotating 'mm' slot to avoid deadlocking against the qb-long
            # accumulator tiles.
            if pre:
                ps_qk = ps_acc.tile([P, 512], F32, tag="acc", name="ps_qk")
            else:
                ps_qk = ps_mm.tile([P, 512], F32, tag="mm", name="ps_qk")
            for kd in range(KD):
                nc.tensor.matmul(
                    ps_qk[:],
                    lhsT=wqk_sb[:, kd, ct * P:(ct + 1) * P],
                    rhs=xT_sb[:, n, kd, :],
                    start=(kd == 0),
                    stop=(kd == KD - 1),
                )
            nc.vector.tensor_scalar_add(
                qkT_sb[:, ct, n * 512:(n + 1) * 512], ps_qk[:],
                bqk_sb[:, ct:ct + 1],
            )

        # PE warm-up: dummy matmuls with no input deps run while the input
        # DMAs land, lifting the HAM clock gate to 8/8 before real work
        warm_sb = mp.tile([P, 512], BF16)
        nc.vector.memset(warm_sb[:], 1.0)
        ps_warm = ps_s.tile([P, 4 * 512], F32, name="ps_warm", tag="sc")
        for w in range(48):
            nc.tensor.matmul(
                ps_warm[:, (w % 4) * 512:(w % 4 + 1) * 512],
                lhsT=warm_sb[:, 0:P],
                rhs=warm_sb[:, :],
                start=True,
                stop=True,
            )

        for ct in (2, 0, 3, 1):   # k,q of head-pair 0 first: earliest expA
            emit_qk(0, ct, pre=True)

        # ---------------- attention + AG + proj, per query block ----------------
        v_sb = mp.tile([P, KT, DL], BF16)
        outT_sb = mp.tile([P, 2, S], BF16)   # pair j: heads 2j (p<64), 2j+1
        g_tiles = [None] * QB

        _vpair = [None]

        def emit_v(tt):
            if tt % 2 == 0:
                _vpair[0] = ps_mm.tile([P, 512], F32, tag="mm", name="ps_v")
            half = (tt % 2) * DL
            ps_v = _vpair[0]
            for kd in range(KD):
                nc.tensor.matmul(
                    ps_v[:, half:half + DL],
                    lhsT=xT_sb[:, tt // 4, kd, (tt % 4) * P:(tt % 4 + 1) * P],
                    rhs=wv_sb[:, kd, :],
                    start=(kd == 0),
                    stop=(kd == KD - 1),
                )
            nc.vector.tensor_copy(v_sb[:, tt, :], ps_v[:, half:half + DL])

        def emit_proj_half(qb, j):
            qs = qb * QW
            g_sb = g_tiles[qb]
            ps_y = ps_mm.tile([P, 512], F32, tag="mm", name="ps_y")
            for kd in range(KD):
                nc.tensor.matmul(
                    ps_y[:],
                    lhsT=wp_sb[:, kd, j * P:(j + 1) * P],
                    rhs=g_sb[:, kd, :],
                    start=(kd == 0),
                    stop=(kd == KD - 1),
                )
            y_sb = yp.tile([P, 512], F32, name="y_sb")
            nc.vector.tensor_scalar_add(y_sb[:], ps_y[:], beff_sb[:, j:j + 1])
            nc.sync.dma_start(yT[j * P:(j + 1) * P, qs:qs + QW], y_sb[:])

        def emit_av_pair(kt, e_sb, ps_av, pair):
            for hh in range(2):
                h = 2 * pair + hh
                nc.tensor.matmul(
                    ps_av[64 * hh:64 * hh + HD, :],
                    lhsT=v_sb[:, kt, h * HD:(h + 1) * HD],
                    rhs=e_sb[:, h * 512:(h + 1) * 512],
                    start=(kt == 0),
                    stop=(kt == KT - 1),
                )

        def emit_rs(kt, e_sb, ps_rs):
            for h in range(HL):
                nc.tensor.matmul(
                    ps_rs[32 * h:32 * h + 1, :],
                    lhsT=ones_sb[:, 0:1],
                    rhs=e_sb[:, h * 512:(h + 1) * 512],
                    start=(kt == 0),
                    stop=(kt == KT - 1),
                    tile_position=(0, 32 * h),
                )

        def make_norm_pair(qb, j, o_sb, r_sb):
            qs = qb * QW

            def _norm():
                rb_ps = ps_mm.tile([P, 512], F32, tag="mm", name="rb_ps")
                for hh in range(2):
                    h = 2 * j + hh
                    nc.tensor.matmul(
                        rb_ps[64 * hh:64 * hh + 64, :],
                        lhsT=onesf_sb[32 * h:32 * h + 1, :],
                        rhs=r_sb[32 * h:32 * h + 1, :],
                        start=True,
                        stop=True,
                        tile_position=(32 * h, 64 * hh),
                    )
                nc.vector.tensor_mul(outT_sb[:, j, qs:qs + QW], o_sb[:],
                                     rb_ps[:])
            return _norm

        def make_ag(qb):
            qs = qb * QW

            def _ag():
                cc_in = dp.tile([2 * P, QW], BF16, name="cc_in")
                nc.sync.dma_start(cc_in[:, :].rearrange("(j p) t -> p j t", p=P),
                                  outT_sb[:, :, qs:qs + QW])
                cc_out = dp.tile([D, QW], BF16, name="cc_out")
                nc.gpsimd.collective_compute(
                    "AllGather",
                    mybir.AluOpType.bypass,
                    replica_groups=GROUPS,
                    ins=[cc_in[:, :].opt()],
                    outs=[cc_out[:, :].opt()],
                )
                g_sb = gp.tile([P, KD, QW], BF16, name="g_sb")
                nc.sync.dma_start(
                    g_sb[:], cc_out[:, :].rearrange("(kd p) t -> p kd t", p=P))
                g_tiles[qb] = g_sb
            return _ag

        # Deadline-scheduled PE filler for each (qb, kt) iteration:
        #  - qb0 carries the remaining qk blocks (k tiles via the acc pool
        #    before the lag-3 accumulators are allocated) and all v tiles
        #  - qb>=1 carry the q blocks for later qbs, the normalization +
        #    AllGather of qb-1 (kt1/kt2), and proj of qb-1 (kt12/kt14)
        filler = {qb: {} for qb in range(QB)}

        def _add(qb, kt, fn):
            filler[qb].setdefault(kt, []).append(fn)

        _add(0, 0, lambda: emit_qk(1, 2, pre=True))
        _add(0, 1, lambda: emit_qk(1, 3, pre=True))
        _add(0, 1, lambda: emit_qk(2, 2, pre=True))
        _add(0, 2, lambda: emit_qk(2, 3, pre=True))
        _add(0, 2, lambda: emit_qk(3, 2, pre=True))
        _add(0, 2, lambda: emit_qk(3, 3, pre=True))
        _v_sched = {_t: [_t] for _t in range(12)}
        _v_sched[11] = [11, 12]
        _v_sched[12] = [13, 14]
        _v_sched[13] = [15]
        for _kt, _ts in _v_sched.items():
            for _t in _ts:
                _add(0, _kt, lambda t=_t: emit_v(t))
        _add(0, 9, lambda: emit_qk(1, 0))
        _add(0, 12, lambda: emit_qk(1, 1))
        _add(1, 2, lambda: emit_qk(2, 0))
        _add(1, 5, lambda: emit_qk(2, 1))
        _add(2, 2, lambda: emit_qk(3, 0))
        _add(2, 5, lambda: emit_qk(3, 1))
        for _qb in (2, 3):
            _add(_qb, 8, lambda q=_qb: emit_proj_half(q - 2, 0))
            _add(_qb, 10, lambda q=_qb: emit_proj_half(q - 2, 1))

        def emit_scores_pair(ps_sc, qb, kt, pair):
            qs = qb * QW
            for hh in range(2):
                h = 2 * pair + hh
                hp = (HD * h) % P                 # 0, 64, 0, 64
                hc = h // 2                       # q ctile; k ctile = 2 + hc
                nc.tensor.matmul(
                    ps_sc[:, h * 512:(h + 1) * 512],
                    lhsT=qkT_sb[hp:hp + HD, 2 + hc, kt * P:(kt + 1) * P],
                    rhs=qkT_sb[hp:hp + HD, hc, qs:qs + QW],
                    start=True,
                    stop=True,
                )

        sc_next = None
        for qb in range(QB):
            qs = qb * QW
            lag = 3 if qb == 0 else 0
            ps_av0 = ps_av1 = ps_rs = None
            pending = []

            if sc_next is None:
                sc_next = ps_s.tile([P, 4 * 512], F32, name="ps_sc", tag="sc")
                emit_scores_pair(sc_next, qb, 0, 0)
                emit_scores_pair(sc_next, qb, 0, 1)
            sc_cur = sc_next
            sc_next = None

            for kt in range(KT):
                e_sb = ep.tile([P, 4 * 512], BF16, name="e_sb")
                nc.scalar.activation(e_sb[:, 0:1024], sc_cur[:, 0:1024], exp_fn,
                                     scale=SCALE)
                nc.scalar.activation(e_sb[:, 1024:2048], sc_cur[:, 1024:2048],
                                     exp_fn, scale=SCALE)
                nxt = kt + 1 < KT
                if nxt:
                    sc_nx = ps_s.tile([P, 4 * 512], F32, name="ps_sc", tag="sc")
                if lag == 0:
                    if ps_av0 is None:
                        ps_av0 = ps_acc.tile([P, 512], F32, tag="acc",
                                             name="ps_av0")
                        ps_av1 = ps_acc.tile([P, 512], F32, tag="acc",
                                             name="ps_av1")
                        ps_rs = ps_acc.tile([P, 512], F32, tag="acc",
                                            name="ps_rs")
                    if nxt:
                        emit_scores_pair(sc_nx, qb, kt + 1, 0)
                    emit_av_pair(kt, e_sb, ps_av0, 0)
                    if nxt:
                        emit_scores_pair(sc_nx, qb, kt + 1, 1)
                    emit_av_pair(kt, e_sb, ps_av1, 1)
                    emit_rs(kt, e_sb, ps_rs)
                else:
                    if nxt:
                        emit_scores_pair(sc_nx, qb, kt + 1, 0)
                        emit_scores_pair(sc_nx, qb, kt + 1, 1)
                    pending.append((kt, e_sb))
                    if len(pending) > lag:
                        if ps_av0 is None:
                            ps_av0 = ps_acc.tile([P, 512], F32, tag="acc",
                                                 name="ps_av0")
                            ps_av1 = ps_acc.tile([P, 512], F32, tag="acc",
                                                 name="ps_av1")
                            ps_rs = ps_acc.tile([P, 512], F32, tag="acc",
                                                name="ps_rs")
                        k0, e0 = pending.pop(0)
                        emit_av_pair(k0, e0, ps_av0, 0)
                        emit_av_pair(k0, e0, ps_av1, 1)
                        emit_rs(k0, e0, ps_rs)
                for fn in filler[qb].get(kt, ()):
                    fn()
                if nxt:
                    sc_cur = sc_nx
            for k0, e0 in pending:
                emit_av_pair(k0, e0, ps_av0, 0)
                emit_av_pair(k0, e0, ps_av1, 1)
                emit_rs(k0, e0, ps_rs)

            if qb + 1 < QB:
                sc_next = ps_s.tile([P, 4 * 512], F32, name="ps_sc", tag="sc")
                emit_scores_pair(sc_next, qb + 1, 0, 0)
                emit_scores_pair(sc_next, qb + 1, 0, 1)

            # release the accumulator PSUM slots fast: raw copies to SBUF
            o_sb = [rp.tile([P, 512], BF16, name="o0_sb"),
                    rp.tile([P, 512], BF16, name="o1_sb")]
            nc.vector.tensor_copy(o_sb[0][:], ps_av0[:])
            nc.vector.tensor_copy(o_sb[1][:], ps_av1[:])
            r_sb = rp.tile([P, 512], F32, name="r_sb")
            nc.vector.reciprocal(r_sb[:], ps_rs[:])   # rows 0/32/64/96 valid

            if qb < QB - 1:
                _add(qb + 1, 1, make_norm_pair(qb, 0, o_sb[0], r_sb))
                _add(qb + 1, 2, make_norm_pair(qb, 1, o_sb[1], r_sb))
                _add(qb + 1, 2, make_ag(qb))
            else:
                make_norm_pair(qb, 0, o_sb[0], r_sb)()
                make_norm_pair(qb, 1, o_sb[1], r_sb)()
                make_ag(qb)()

        emit_proj_half(QB - 2, 0)
        emit_proj_half(QB - 2, 1)
        emit_proj_half(QB - 1, 0)
        emit_proj_half(QB - 1, 1)


def _build():
    if "nc" in _CACHE:
        return _CACHE["nc"]
    nc = bacc.Bacc(
        "TRN2",
        target_bir_lowering=False,
        debug=False,
        num_devices=NCORES,
    )
    xT = nc.declare_dram_parameter("xT", [QB, P, KD * 512], BF16, isOutput=False)
    wqk = nc.declare_dram_parameter("wqk", [P, KD * 2 * DL], BF16, isOutput=False)
    wv = nc.declare_dram_parameter("wv", [P, KD * DL], BF16, isOutput=False)
    wp = nc.declare_dram_parameter("wp", [P, KD * DL], BF16, isOutput=False)
    bqk = nc.declare_dram_parameter("bqk", [P, 4], F32, isOutput=False)
    beff = nc.declare_dram_parameter("beff", [P, 2], F32, isOutput=False)
    yT = nc.declare_dram_parameter("yT", [DL, S], F32, isOutput=True)

    with tile.TileContext(nc) as tc:
        _emit(nc, tc, xT, wqk, wv, wp, bqk, beff, yT)
    nc.compile()
    _CACHE["nc"] = nc
    return nc


def kernel(x, W_qkv, b_qkv, W_proj, b_proj):
    x = np.asarray(x, np.float32)
    W_qkv = np.asarray(W_qkv, np.float32)
    b_qkv = np.asarray(b_qkv, np.float32)
    W_proj = np.asarray(W_proj, np.float32)
    b_proj = np.asarray(b_proj, np.float32)

    nc = _build()

    b_v = b_qkv[2 * D:3 * D]
    xTt = {}
    for b in range(B):
        xT = np.ascontiguousarray(x[b].T)            # [1024, 2048]
        t = xT.reshape(KD, P, QB, 512).transpose(2, 1, 0, 3)
        xTt[b] = np.ascontiguousarray(t.reshape(QB, P, KD * 512)).astype(NBF16)

    in_maps = []
    for c in range(NCORES):
        b, g = divmod(c, 4)
        cs = DL * g
        wqk_c = np.concatenate(
            [W_qkv[:, cs:cs + DL], W_qkv[:, D + cs:D + cs + DL]], axis=1)
        bqk_c = np.concatenate(
            [b_qkv[cs:cs + DL], b_qkv[D + cs:D + cs + DL]]).reshape(4, P).T
        beff_c = (b_v @ W_proj[:, cs:cs + DL] + b_proj[cs:cs + DL]).reshape(2, P).T
        in_maps.append({
            "xT": xTt[b],
            "wqk": _restripe(wqk_c).astype(NBF16),
            "wv": _restripe(W_qkv[:, 2 * D + cs:2 * D + cs + DL]).astype(NBF16),
            "wp": _restripe(W_proj[:, cs:cs + DL]).astype(NBF16),
            "bqk": np.ascontiguousarray(bqk_c, np.float32),
            "beff": np.ascontiguousarray(beff_c, np.float32),
        })

    trace = bool(int(os.environ.get("TRN_KERNEL_TRACE", "0")))
    res = run_bass_kernel_spmd(nc, in_maps, core_ids=list(range(NCORES)),
                               trace=trace)
    if trace and res.exec_time_ns is not None:
        print(f"HW exec time: {res.exec_time_ns} ns", flush=True)
    _CACHE["last_result"] = res

    out = np.empty((B, S, D), np.float32)
    for c in range(NCORES):
        b, g = divmod(c, 4)
        out[b, :, DL * g:DL * (g + 1)] = res.results[c]["yT"].T
    return out
